# revision 30
# baseline (speedup 1.0000x reference)
"""Trainium2 Bass kernel: causal attention with 3D (Rodrigues) RoPE.

Sharding: tensor-parallel over heads (2 heads/core on 8 cores) for
QKV projection + RoPE + SDPA, then an AllToAll redistributes attention
outputs so the output projection is sharded over tokens (512/core).

Schedule (vs baseline, ~399us -> ~300us):
  - all matmuls in bf16 (same PE cycles/row as f32r on TRN2, half the
    DMA/SBUF traffic)
  - V computed directly in [token, head-dim] layout via flipped
    matmuls (lhsT = x tile): kills the 64 PE transposes and the
    64-row straddle M-tile (-34k PE row-streams)
  - head-0 attention finely woven into the projection sweep one chunk
    behind: projection M-groups and V-groups are emitted as units
    between every 2 attention QK tiles, so the PE chews projection
    matmuls during every softmax-exp wait (ACT is the attention
    bottleneck) and PVs drain at full rate afterwards; exps precede
    next-chunk evictions on the in-order ACT queue
  - head-1 attention entirely post-sweep: its ~30us of ACT-bound exp
    covers AllToAll#1; o-proj half0 covers part of AllToAll#2
  - engine split: PE matmuls only; ACT exp + 3 large evict pieces;
    DVE 6 small evicts + rope muls/adds + v-evict + normalize; shifts
    via SW-DGE DMAs off the Pool queue; Pool also does the
    1/denom partition broadcast
  - single bulk DMAs (x chunk [128,12,512], cco, woT halves
    [128,6,1536]), wall split for fast PE start
  - AllToAll payloads bf16; tiny warm-up AllToAll absorbs the ~11.5us
    first-collective setup; dummy matmuls hold the PE p-state (2.4GHz
    needs 3us continuous busy) across the AllToAll#2 wait; output
    written bf16 and upcast on host
"""

import sys

sys.path.insert(0, "/opt/trn_rl_repo")

import numpy as np

D_MODEL, N_HEADS, HEAD_DIM, MAX_POS = 1536, 16, 96, 4096
B, T = 2, 2048
NTOK = B * T                      # 4096
NCORES = 8
HPC = N_HEADS // NCORES           # 2 heads per core
NTRIP = HEAD_DIM // 3             # 32 triplets
KT = D_MODEL // 128               # 12 contraction tiles
NCH = NTOK // 512                 # 8 token chunks of 512
TQC = T // 512                    # 4 query chunks per batch
SCALE = 1.0 / np.sqrt(HEAD_DIM)

_CACHE = {}


def _build_nc():
    import concourse.bass as bass
    import concourse.mybir as mybir
    import concourse.tile as tile
    from concourse import bacc

    f32 = mybir.dt.float32
    f32r = mybir.dt.float32r
    bf16 = mybir.dt.bfloat16
    MUL = mybir.AluOpType.mult
    ADD = mybir.AluOpType.add
    CP = mybir.ActivationFunctionType.Copy
    EXP = mybir.ActivationFunctionType.Exp

    nc = bacc.Bacc("TRN2", target_bir_lowering=False, debug=False,
                   enable_asserts=False, num_devices=NCORES)

    xT = nc.dram_tensor("xT", [D_MODEL, NTOK], bf16, kind="ExternalInput").ap()
    wallT = nc.dram_tensor("wallT", [D_MODEL, 576], bf16,
                           kind="ExternalInput").ap()
    woT = nc.dram_tensor("woT", [D_MODEL, D_MODEL], bf16,
                         kind="ExternalInput").ap()
    cco = nc.dram_tensor("cco", [96, 3, T], bf16, kind="ExternalInput").ap()
    msk = nc.dram_tensor("msk", [128, 128], bf16, kind="ExternalInput").ap()
    out = nc.dram_tensor("out", [D_MODEL, 512], bf16,
                         kind="ExternalOutput").ap()

    xTr = xT.rearrange("(j p) t -> p j t", p=128)      # [128, 12, 4096]
    wallTr = wallT.rearrange("(j p) c -> p j c", p=128)  # [128, 12, 576]
    woTr = woT.rearrange("(j p) c -> p j c", p=128)    # [128, 12, 1536]

    with tile.TileContext(nc) as tc:
        with tc.tile_pool(name="dram", bufs=1, space="DRAM") as dram:
            a2a_in = [dram.tile([NCH, 96, 512], bf16, name=f"a2a_in{h}")
                      for h in range(HPC)]
            a2a_out = [dram.tile([NCH, 96, 512], bf16, name=f"a2a_out{h}")
                       for h in range(HPC)]

            with tc.tile_pool(name="pp", bufs=1) as pp:
                qk_rot = [pp.tile([96, NTOK], bf16, tag=f"qkrot{i}",
                                  name=f"qkrot{i}") for i in range(4)]
                v_sb = pp.tile([128, NTOK // 128, 194], bf16, tag="vsb")
                m_sb = pp.tile([128, 128], bf16, tag="msb")
                cco_sb = pp.tile([96, 3, T], bf16, tag="cco")
                wall_sb = pp.tile([128, KT, 576], bf16, tag="wall")

                # ones columns of V never change: set them once
                nc.vector.memset(v_sb[:, :, 96:97], 1.0)
                nc.vector.memset(v_sb[:, :, 193:194], 1.0)
                # scratch operands for PE p-state warm-up at t=0
                wu_l = pp.tile([128, 128], bf16, tag="wul")
                wu_r = pp.tile([128, 512], bf16, tag="wur")
                nc.vector.memset(wu_l[:], 0.0)
                nc.vector.memset(wu_r[:], 0.0)

                def attn_block(h, b, cl, ps_s, ps_pv, pts, pbs,
                               gp_after=None, units=None,
                               drain_units=True, pace_pv=False):
                    """SDPA for one (head, batch, 512-query block).

                    units: projection-work closures woven between QK tiles
                    so the PE has independent matmuls while ACT catches up
                    on exps; PVs drain afterwards at full rate.
                    """
                    qoff = b * T + cl * 512
                    pv = ps_pv.tile([128, 512], f32, tag="pv", name="pv")
                    ntk = 4 * cl + 4
                    order = (list(range(4 * cl, ntk)) + list(range(4 * cl)))
                    pend = []

                    def emit_pv():
                        ti, tt, lo, pt = pend.pop(0)
                        nc.tensor.matmul(
                            pv[0:97, lo:512],
                            v_sb[:, b * 16 + tt, h * 97:h * 97 + 97],
                            pt[:, lo:512], start=(ti == 0),
                            stop=(ti == ntk - 1), skip_group_check=True)

                    for ti, tt in enumerate(order):
                        koff = b * T + tt * 128
                        # diagonal tiles: columns < lo fully masked -> skip
                        lo = (tt - 4 * cl) * 128 if tt >= 4 * cl else 0
                        sp = ps_s.tile([128, 512], f32, tag="s", name="sp")
                        nc.tensor.matmul(
                            sp[:, lo:512],
                            qk_rot[2 + h][:, koff:koff + 128],
                            qk_rot[h][:, qoff + lo:qoff + 512],
                            start=True, stop=True)
                        pt = pts.tile([128, 512], bf16, tag="p", name="pt")
                        nc.scalar.activation(pt[:, lo:512], sp[:, lo:512], EXP)
                        if tt >= 4 * cl:
                            nc.vector.tensor_tensor(
                                pt[:, lo:lo + 128], pt[:, lo:lo + 128],
                                m_sb[:], MUL)
                        pend.append((ti, tt, lo, pt))
                        if units is None:
                            if len(pend) > 2:
                                emit_pv()
                        else:
                            if pace_pv and len(pend) > 2:
                                emit_pv()
                            if ti % 2 == 1 and units:
                                units.pop(0)()
                    if drain_units:
                        while units:
                            units.pop(0)()
                    while pend:
                        emit_pv()
                    # normalize: 1/denom broadcast across partitions (Pool).
                    # custom-DVE reciprocal can't read PSUM: stage to SBUF.
                    lcp = pbs.tile([1, 512], f32, tag="lcp", name="lcp")
                    nc.vector.tensor_copy(lcp[:], pv[96:97, :])
                    linv = pbs.tile([1, 512], f32, tag="linv", name="linv")
                    nc.vector.reciprocal_approx_fast(linv[:], lcp[:])
                    brow = pbs.tile([96, 512], f32, tag="brow", name="brow")
                    bc = nc.gpsimd.partition_broadcast(brow[:], linv[:])
                    if gp_after is not None:
                        tile.add_dep_helper(bc.ins, gp_after.ins, sync=False,
                                            reason="gpsimd queue order")
                    att = pbs.tile([96, 512], bf16, tag="att", name="att")
                    nc.vector.tensor_tensor(att[:], pv[0:96, :], brow[:],
                                            MUL)
                    return nc.sync.dma_start(a2a_in[h][b * TQC + cl, :, :],
                                             att[:])

                # warm up the collective ring early so A2A#1 starts fast
                cc_warm = [dram.tile([NCH, 64], bf16, name=f"ccw{i}")
                           for i in range(2)]
                with tc.high_priority():
                    nc.gpsimd.collective_compute(
                        "AllToAll", mybir.AluOpType.bypass,
                        replica_groups=[list(range(NCORES))],
                        ins=[cc_warm[0].opt()], outs=[cc_warm[1].opt()])

                def mk_warm(ps_pool, n):
                    wt = ps_pool.tile([128, 512], f32, tag="s",
                                      name="wpace")

                    def f():
                        nc.tensor.matmul(wt[:], wu_l[:], wu_r[:],
                                         start=True, stop=True,
                                         skip_group_check=True)
                    return [f] * n

                # ---------------- sweep: projection + rope + attention ------
                last_h0_w = [None]
                with tc.tile_pool(name="pxt", bufs=2) as pxt, \
                     tc.tile_pool(name="praw", bufs=2) as praw, \
                     tc.tile_pool(name="pts", bufs=18) as pts, \
                     tc.tile_pool(name="pbs", bufs=3) as pbs, \
                     tc.tile_pool(name="ps_qk", bufs=2, space="PSUM") as ps_qk, \
                     tc.tile_pool(name="ps_v", bufs=2, space="PSUM") as ps_v, \
                     tc.tile_pool(name="ps_s", bufs=2, space="PSUM") as ps_s, \
                     tc.tile_pool(name="ps_pv", bufs=2, space="PSUM") as ps_pv:

                    def load_x(ch):
                        t = pxt.tile([128, KT, 512], bf16, tag="xt",
                                     name=f"xt{ch}")
                        nc.sync.dma_start(
                            t[:], xTr[:, :, ch * 512:(ch + 1) * 512])
                        return t

                    nc.sync.dma_start(wall_sb[:, 0:1, :],
                                      wallTr[:, 0:1, :])
                    first_x = pxt.tile([128, KT, 512], bf16, tag="xt",
                                       name="xt0")
                    nc.sync.dma_start(first_x[:, 0:4, :], xTr[:, 0:4, 0:512])
                    nc.sync.dma_start(wall_sb[:, 1:6, :], wallTr[:, 1:6, :])
                    nc.scalar.dma_start(first_x[:, 4:12, :],
                                        xTr[:, 4:12, 0:512])
                    nc.sync.dma_start(wall_sb[:, 6:12, :], wallTr[:, 6:12, :])
                    nc.sync.dma_start(cco_sb[:], cco[:])
                    nc.sync.dma_start(m_sb[:], msk[:])
                    # ramp the PE p-state while the first loads land: the
                    # first real matmul then starts at a hot clock
                    wq = ps_s.tile([128, 512], f32, tag="s", name="warm0")
                    for wi in range(14):
                        nc.tensor.matmul(wq[:], wu_l[:], wu_r[:],
                                         start=True, stop=True,
                                         skip_group_check=True)

                    def emit_qk_group(m, xt, pq_list):
                        # one of 3 M-tiles over rows q0 q1 k0 k1 (384)
                        ps = ps_qk.tile([128, 512], f32, tag="qk",
                                        name=f"pq{m}")
                        for kt in range(KT):
                            nc.tensor.matmul(
                                ps[:],
                                wall_sb[:, kt, m * 128:(m + 1) * 128],
                                xt[:, kt, :], start=(kt == 0),
                                stop=(kt == KT - 1))
                        pq_list.append(ps)

                    def emit_v_group(ts_, ch, xt, pvb_box):
                        # v directly in [token, 2*96] layout
                        if ts_ % 2 == 0:
                            pvb_box[ts_ // 2] = ps_v.tile(
                                [128, 2, 192], f32, tag="v",
                                name=f"pvb{ts_ // 2}")
                        dstv = pvb_box[ts_ // 2][:, ts_ % 2, :]
                        for kt in range(KT):
                            nc.tensor.matmul(
                                dstv,
                                xt[:, kt, ts_ * 128:(ts_ + 1) * 128],
                                wall_sb[:, kt, 384:576],
                                start=(kt == 0), stop=(kt == KT - 1))
                        if ts_ % 2 == 1:
                            # v eviction (DVE; Pool has no PSUM access)
                            half = ts_ // 2
                            g = ch * 4 + half * 2
                            for hh in range(HPC):
                                nc.vector.tensor_copy(
                                    v_sb[:, g:g + 2, hh * 97:hh * 97 + 96],
                                    pvb_box[half][:, :,
                                                  hh * 96:(hh + 1) * 96])

                    def evict_rope(ch, pq):
                        coff = (ch % TQC) * 512
                        # qk eviction PSUM f32 -> SBUF bf16 (ACT)
                        raws = [praw.tile([96, 512], bf16, tag=f"raw{m}",
                                          name=f"raw{m}") for m in range(4)]
                        ev_act = [
                            (raws[0][0:96, :], pq[0][0:96, :]),
                            (raws[1][0:32, :], pq[0][96:128, :]),
                            (raws[2][0:64, :], pq[1][64:128, :]),
                        ]
                        ev_dve = [
                            (raws[1][32:64, :], pq[1][0:32, :]),
                            (raws[1][64:96, :], pq[1][32:64, :]),
                            (raws[2][64:96, :], pq[2][0:32, :]),
                            (raws[3][0:32, :], pq[2][32:64, :]),
                            (raws[3][32:64, :], pq[2][64:96, :]),
                            (raws[3][64:96, :], pq[2][96:128, :]),
                        ]
                        for dst_ap, src_ap in ev_act:
                            nc.scalar.activation(dst_ap, src_ap, CP)
                        for dst_ap, src_ap in ev_dve:
                            nc.vector.tensor_copy(dst_ap, src_ap)
                        # rope: dst = raw*c0 + rot32(raw)*c1 + rot64(raw)*c2
                        # shifts as 32-partition quadrant-aligned DVE copies
                        for m in range(4):
                            raw = raws[m]
                            g1 = praw.tile([96, 512], bf16, tag=f"g1_{m}",
                                           name=f"g1_{m}")
                            g2 = praw.tile([96, 512], bf16, tag=f"g2_{m}",
                                           name=f"g2_{m}")
                            nc.sync.dma_start(g1[0:64, :], raw[32:96, :])
                            nc.sync.dma_start(g1[64:96, :], raw[0:32, :])
                            nc.gpsimd.dma_start(g2[0:32, :], raw[64:96, :])
                            nc.gpsimd.dma_start(g2[32:96, :], raw[0:64, :])
                            dst = qk_rot[m][:, ch * 512:(ch + 1) * 512]
                            nc.vector.tensor_tensor(
                                dst, raw[:], cco_sb[:, 0, coff:coff + 512],
                                MUL)
                            t1 = praw.tile([96, 512], bf16, tag=f"t1_{m}",
                                           name=f"t1_{m}")
                            nc.vector.tensor_tensor(
                                t1[:], g1[:], cco_sb[:, 1, coff:coff + 512],
                                MUL)
                            nc.vector.tensor_tensor(dst, dst, t1[:], ADD)
                            t2 = praw.tile([96, 512], bf16, tag=f"t2_{m}",
                                           name=f"t2_{m}")
                            nc.vector.tensor_tensor(
                                t2[:], g2[:], cco_sb[:, 2, coff:coff + 512],
                                MUL)
                            nc.vector.tensor_tensor(dst, dst, t2[:], ADD)

                    xt_cur = first_x
                    for ch in range(NCH):
                        xt_next = load_x(ch + 1) if ch + 1 < NCH else None
                        pq_list = []
                        pvb_box = {}
                        emit_qk_group(0, xt_cur, pq_list)
                        if ch > 0:
                            b, cl = (ch - 1) // TQC, (ch - 1) % TQC

                            def mku(f, *a):
                                return lambda: f(*a)

                            units = [
                                mku(emit_qk_group, 1, xt_cur, pq_list),
                                mku(emit_qk_group, 2, xt_cur, pq_list),
                                lambda pl=pq_list, c=ch: evict_rope(c, pl),
                                mku(emit_v_group, 0, ch, xt_cur, pvb_box),
                                mku(emit_v_group, 1, ch, xt_cur, pvb_box),
                                mku(emit_v_group, 2, ch, xt_cur, pvb_box),
                                mku(emit_v_group, 3, ch, xt_cur, pvb_box),
                            ]
                            w = attn_block(0, b, cl, ps_s, ps_pv, pts, pbs,
                                           units=units,
                                           drain_units=(cl > 1))
                            last_h0_w[0] = w
                            if cl <= 1:
                                attn_block(1, b, cl, ps_s, ps_pv, pts, pbs,
                                           units=units)
                        else:
                            emit_qk_group(1, xt_cur, pq_list)
                            emit_qk_group(2, xt_cur, pq_list)
                            evict_rope(ch, pq_list)
                            for ts_ in range(4):
                                emit_v_group(ts_, ch, xt_cur, pvb_box)
                        xt_cur = xt_next
                    b, cl = (NCH - 1) // TQC, (NCH - 1) % TQC
                    last_h0_w[0] = attn_block(0, b, cl, ps_s, ps_pv,
                                              pts, pbs,
                                              units=mk_warm(ps_s, 8),
                                              pace_pv=True)

                # ---------------- A2A#1 + head-1 late blocks + oproj --------
                with tc.high_priority():
                    trig1 = nc.gpsimd.collective_compute(
                        "AllToAll", mybir.AluOpType.bypass,
                        replica_groups=[list(range(NCORES))],
                        ins=[a2a_in[0].opt()], outs=[a2a_out[0].opt()])

                with tc.tile_pool(name="ptl", bufs=1) as ptl, \
                     tc.tile_pool(name="pts2", bufs=6) as pts2, \
                     tc.tile_pool(name="pbs2", bufs=3) as pbs2:
                    # preload o-proj weights on the ACT hwdge queue (sync
                    # queue keeps streaming a2a_in writes)
                    wot_sb = [ptl.tile([128, 6, D_MODEL], bf16,
                                       tag=f"wot{hf}", name=f"wot{hf}")
                              for hf in range(2)]
                    for hf in range(2):
                        nc.scalar.dma_start(
                            wot_sb[hf][:], woTr[:, hf * 6:(hf + 1) * 6, :])

                    last_w = None
                    with tc.tile_pool(name="ps_s2", bufs=4,
                                      space="PSUM") as ps_s2, \
                         tc.tile_pool(name="ps_pv2", bufs=3,
                                      space="PSUM") as ps_pv2:
                        gp = trig1
                        for b in range(B):
                            for cl in (2, 3):
                                last_w = attn_block(1, b, cl, ps_s2, ps_pv2,
                                                    pts2, pbs2, gp_after=gp,
                                                    units=mk_warm(
                                                        ps_s2, 2 * cl + 2),
                                                    pace_pv=True)
                                gp = None
                        # dummy QK matmuls fill the PE-dry stretch while
                        # ACT finishes head-1 exps and AllToAll#1 lands
                        sp_w = ps_s2.tile([128, 512], f32, tag="swarm",
                                          name="sp_w", bufs=1)
                        for wi in range(12):
                            nc.tensor.matmul(
                                sp_w[:],
                                qk_rot[3][:, (wi % 16) * 128:
                                          (wi % 16) * 128 + 128],
                                qk_rot[1][:, 0:512],
                                start=True, stop=True,
                                skip_group_check=True)

                    def load_att2(half, after=None):
                        # split across both HW-DGE queues: these loads sit
                        # on the o-proj critical path after each AllToAll
                        flat = a2a_out[half][:].rearrange("a b c -> (a b) c")
                        att2 = []
                        for et in range(6):
                            t = ptl.tile([128, 512], bf16,
                                         tag=f"att2_{half}_{et}",
                                         name=f"att2_{half}_{et}")
                            eng = nc.sync if et % 2 == 0 else nc.scalar
                            ld = eng.dma_start(
                                t[:], flat[et * 128:(et + 1) * 128, :])
                            if after is not None and et % 2 == 0:
                                tile.add_dep_helper(
                                    ld.ins, after.ins, sync=False,
                                    reason="sync queue order")
                            att2.append(t)
                        return att2

                    att2A = load_att2(0, last_w)
                    with tc.high_priority():
                        nc.gpsimd.collective_compute(
                            "AllToAll", mybir.AluOpType.bypass,
                            replica_groups=[list(range(NCORES))],
                            ins=[a2a_in[1].opt()], outs=[a2a_out[1].opt()])

                    partA = ptl.tile([128, KT, 512], f32, tag="partA")

                    def oproj_tile(half, dt_, att2, ps_o):
                        po = ps_o.tile([128, 512], f32, tag="o",
                                       name=f"po_{half}_{dt_}")
                        for et in range(6):
                            nc.tensor.matmul(
                                po[:],
                                wot_sb[half][:, et,
                                             dt_ * 128:(dt_ + 1) * 128],
                                att2[et][:], start=(et == 0),
                                stop=(et == 5), skip_group_check=True)
                        if half == 0:
                            nc.scalar.activation(
                                partA[:, dt_, :], po[:], CP)
                        else:
                            ot = pbs2.tile([128, 512], bf16, tag="ot",
                                           name="ot")
                            nc.vector.tensor_tensor(
                                ot[:], po[:], partA[:, dt_, :], ADD)
                            eng = nc.sync if dt_ % 2 == 0 else nc.scalar
                            eng.dma_start(
                                out[dt_ * 128:(dt_ + 1) * 128, :], ot[:])

                    with tc.tile_pool(name="ps_o", bufs=7,
                                      space="PSUM") as ps_o:
                        for dt_ in range(KT):
                            oproj_tile(0, dt_, att2A, ps_o)
                        att2B = load_att2(1)
                        # dummy matmuls keep the PE p-state hot while the
                        # second AllToAll lands
                        warm = ps_o.tile([128, 512], f32, tag="warm",
                                         name="warm", bufs=1)
                        for wi in range(14):
                            nc.tensor.matmul(
                                warm[:], wot_sb[0][:, wi % 6, 0:128],
                                att2A[wi % 6][:], start=True, stop=True,
                                skip_group_check=True)
                        for dt_ in range(KT):
                            oproj_tile(1, dt_, att2B, ps_o)

    nc.compile()
    return nc


def _plane_major(w):
    """Reorder head-dim rows 3k+i -> 32i+k (per 96-row head block)."""
    idx = np.empty(96, dtype=np.int64)
    for i in range(3):
        for k in range(NTRIP):
            idx[32 * i + k] = 3 * k + i
    return w[idx]


def _prep_inputs(x, w_qkv, w_o, Rs):
    import ml_dtypes
    bf = ml_dtypes.bfloat16

    x = np.asarray(x, dtype=np.float32)
    w_qkv = np.asarray(w_qkv, dtype=np.float32)
    w_o = np.asarray(w_o, dtype=np.float32)
    Rs = np.asarray(Rs, dtype=np.float32)

    xT = np.ascontiguousarray(x.reshape(NTOK, D_MODEL).T).astype(bf)

    # rope coefficients, plane-major rows: C[d, delta, t]
    R = Rs[:T]                                   # (T, 32, 3, 3)
    cco = np.empty((96, 3, T), dtype=np.float32)
    for d in range(3):
        for i in range(3):
            cco[32 * i:32 * i + 32, d, :] = R[:, :, i, (i + d) % 3].T
    cco = cco.astype(bf)

    # lower-triangular mask for the mixed 128x128 diagonal block
    j = np.arange(128)[:, None]
    i = np.arange(128)[None, :]
    msk = (j <= i).astype(bf)

    # w_o columns-for-even-heads first, then odd (matches split A2A halves)
    woT = np.ascontiguousarray(w_o.T)            # rows e = h*96+d
    perm = np.concatenate(
        [np.arange(h * 96, (h + 1) * 96) for h in range(0, 16, 2)] +
        [np.arange(h * 96, (h + 1) * 96) for h in range(1, 16, 2)])
    woTp = np.ascontiguousarray(woT[perm]).astype(bf)

    def w_row(s, h):
        base = (s * N_HEADS + h) * HEAD_DIM
        return w_qkv[base:base + HEAD_DIM]

    in_maps = []
    for c in range(NCORES):
        h0, h1 = 2 * c, 2 * c + 1
        wall = np.concatenate([
            _plane_major(w_row(0, h0)) * SCALE,
            _plane_major(w_row(0, h1)) * SCALE,
            _plane_major(w_row(1, h0)),
            _plane_major(w_row(1, h1)),
            w_row(2, h0),
            w_row(2, h1),
        ], axis=0)                               # [576, 1536]
        wallT = np.ascontiguousarray(wall.T).astype(bf)  # [1536, 576]
        in_maps.append({
            "xT": xT, "wallT": wallT, "woT": woTp,
            "cco": cco, "msk": msk,
        })
    return in_maps


def kernel(x, w_qkv, w_o, Rs):
    from concourse import bass_utils

    if "nc" not in _CACHE:
        _CACHE["nc"] = _build_nc()
    nc = _CACHE["nc"]
    in_maps = _prep_inputs(x, w_qkv, w_o, Rs)
    res = bass_utils.run_bass_kernel_spmd(
        nc, in_maps, core_ids=list(range(NCORES)))
    full_T = np.concatenate(
        [np.asarray(res.results[c]["out"], dtype=np.float32)
         for c in range(NCORES)], axis=1)        # [1536, 4096]
    return np.ascontiguousarray(full_T.T).reshape(B, T, D_MODEL)
